# revision 1
# baseline (speedup 1.0000x reference)
"""Bass/Trainium2 kernel for batched dot-product attention.

Problem: q,k,v [B=4, S=4096, D=1024]; projections to dk=dv=128; softmax
attention per batch element.  Sharded over 8 NeuronCores as (batch,
query-half): core c handles batch c//2, queries (c%2)*2048 ... +2048.

All layouts on-chip keep the contraction dimension on SBUF partitions:
  qT/kT/vT   [d_model, seq]   (host pre-transposed, bf16)
  kpT/qpT    [dk, seq]        (projection output, bf16)
  vp         [seq, dv]        (natural layout via PE transpose, bf16)
  S^T tiles  [keys, q]        (scores transposed, PSUM)
  out^T      [dv, q]          (final output transposed; host undoes)

Query blocks are processed in PAIRS (1024-wide exp tiles amortize the
ScalarE per-op overhead and halve AV weight loads).  Pair 0's attention
chunks are interleaved into the projection kb-loop so the TensorE stays
busy while kT/vT stream in.  Softmax denominators (sum over keys =
partition axis) via a ones-vector matmul; normalization via
partition_broadcast + reciprocal + multiply off the critical path.
Scale 1/sqrt(dk) is folded into wq/bq on the host.
"""

import math

import numpy as np
import ml_dtypes

import concourse.bass as bass
import concourse.tile as tile
from concourse import bacc, mybir
from concourse.bass_utils import run_bass_kernel_spmd

B, S, DM, DK, DV = 4, 4096, 1024, 128, 128
N_CORES = 8
SQ = S // 2          # queries per core
NQB = SQ // 512      # query blocks of 512 per core (4)
NKC = S // 128       # key chunks of 128 (32)
NMC = DM // 128      # d_model chunks (8)
NKB = S // 512       # key blocks of 512 (8)

BF16 = mybir.dt.bfloat16
F32 = mybir.dt.float32
F32R = mybir.dt.float32r
NP_BF16 = ml_dtypes.bfloat16

E_DT = BF16          # dtype of exp tiles (AV moving operand)
AV_STAGGER = 2       # pair-chunks the exp/AV drain lags the S matmuls

Identity = mybir.ActivationFunctionType.Identity
Copy = mybir.ActivationFunctionType.Copy
Exp = mybir.ActivationFunctionType.Exp


def _emit(tc: tile.TileContext, aps: dict):
    nc = tc.nc
    qT, kT, vT = aps["qT"], aps["kT"], aps["vT"]
    outT = aps["outT"]

    with tc.tile_pool(name="persist", bufs=1) as persist:
        # --- constants ---
        w_sb = {}
        for name in ("wq", "wk", "wv"):
            t = persist.tile([128, NMC, 128], BF16, tag=f"w_{name}", name=f"w_{name}")
            nc.scalar.dma_start(t[:], aps[name][:])
            w_sb[name] = t
        bias_sb = persist.tile([128, 4], F32, tag="bias")
        nc.scalar.dma_start(bias_sb[:], aps["bias_pack"][:])
        bq_ap, bk_ap, bv_ap = bias_sb[:, 0:1], bias_sb[:, 1:2], bias_sb[:, 2:3]
        ones_ap = bias_sb[:, 3:4]
        ident_sb = persist.tile([128, 128], BF16, tag="ident")
        nc.scalar.dma_start(ident_sb[:], aps["ident"][:])

        # --- persistent activations ---
        kpT_blk = [persist.tile([128, 512], BF16, tag=f"kpT{i}", name=f"kpT{i}")
                   for i in range(NKB)]
        qpT_t = [persist.tile([128, 512], BF16, tag=f"qpT{i}", name=f"qpT{i}")
                 for i in range(NQB)]
        vp_pair = [persist.tile([128, 256], BF16, tag=f"vpp{i}", name=f"vpp{i}")
                   for i in range(NKC // 2)]
        sums_sb = persist.tile([1, SQ], F32, tag="sums", name="sums_sb")

        with (
            tc.tile_pool(name="op", bufs=2, space="PSUM") as op,
            tc.tile_pool(name="pp", bufs=2, space="PSUM") as pp,
            tc.tile_pool(name="sp", bufs=2, space="PSUM") as sp,
            tc.tile_pool(name="xs", bufs=2) as xs,
            tc.tile_pool(name="ep", bufs=3) as ep,
            tc.tile_pool(name="e1p", bufs=1) as e1p,
            tc.tile_pool(name="accp", bufs=2) as accp,
            tc.tile_pool(name="miscp", bufs=2) as miscp,
        ):
            # ---- input fetch + qp projection helpers ----
            kxs, vxs = {}, {}

            def fetch_kx(kb):
                kx = xs.tile([128, NMC, 512], BF16, tag="kx", name=f"kx{kb}",
                             bufs=3)
                nc.sync.dma_start(kx[:], kT[kb])
                kxs[kb] = kx

            def fetch_vx(kb):
                vx = xs.tile([128, NMC, 512], BF16, tag="vx", name=f"vx{kb}",
                             bufs=3)
                nc.sync.dma_start(vx[:], vT[kb])
                vxs[kb] = vx

            qxs = {}

            def fetch_q(qb):
                qx = xs.tile([128, NMC, 512], BF16, tag="qx", name=f"qx{qb}",
                             bufs=3)
                nc.sync.dma_start(qx[:], qT[qb])
                qxs[qb] = qx

            def project_q(qb):
                qx = qxs.pop(qb)
                psq = sp.tile([128, 512], F32, tag="sp", name=f"psq{qb}")
                for c in range(NMC):
                    nc.tensor.matmul(
                        psq[:], lhsT=w_sb["wq"][:, c, :],
                        rhs=qx[:, c, :],
                        start=(c == 0), stop=(c == NMC - 1),
                    )
                nc.vector.tensor_scalar_add(qpT_t[qb][:], psq[:], bq_ap)

            # kx0 leads, then the pair-0 queries, then the rest; qp0/qp1
            # and kp0 are projected before the kb loop so the first score
            # chunks (and the exp chain) start as early as possible
            fetch_kx(0)
            fetch_q(0)
            fetch_q(1)
            fetch_vx(0)
            fetch_kx(1)
            fetch_q(2)
            fetch_q(3)

            def proj_k(kb):
                kx = kxs.pop(kb)
                psk = pp.tile([128, 512], F32, tag="pp", name=f"psk{kb}")
                for c in range(NMC):
                    nc.tensor.matmul(
                        psk[:], lhsT=w_sb["wk"][:, c, :], rhs=kx[:, c, :],
                        start=(c == 0), stop=(c == NMC - 1),
                    )
                nc.vector.tensor_scalar_add(kpT_blk[kb][:], psk[:], bk_ap)

            # ---- attention pair machinery ----
            def pair_begin(pidx, spool, defer_av=False):
                qa, qb_ = 2 * pidx, 2 * pidx + 1
                return dict(
                    p=pidx, qs=(qa, qb_), sp=spool, defer=defer_av,
                    o=None if defer_av else
                      [op.tile([128, 512], F32, tag="op", name=f"o{q}")
                       for q in (qa, qb_)],
                    acc=accp.tile([128, 1024], F32, tag="acc", name=f"acc{pidx}"),
                    pend=[], evs=[],
                )

            def pair_drain(st):
                kc, s = st["pend"].pop(0)
                if st["defer"]:
                    e = e1p.tile([128, 1024], E_DT, tag=f"e1_{kc}",
                                 name=f"e{st['p']}_{kc}")
                else:
                    e = ep.tile([128, 1024], E_DT, tag="e", name=f"e{st['p']}_{kc}")
                nc.scalar.activation(e[:], s[:], Exp)
                if kc % 2 == 0:
                    st["elast"] = e
                else:
                    # one bf16 add level halves the f32 accumulate traffic
                    tmp = ep.tile([128, 1024], BF16, tag="tmp", name=f"t{st['p']}_{kc}")
                    nc.vector.tensor_add(tmp[:], st["elast"][:], e[:])
                    if kc == 1:
                        nc.vector.tensor_copy(st["acc"][:], tmp[:])
                    else:
                        nc.vector.tensor_add(st["acc"][:], st["acc"][:], tmp[:])
                if st["defer"]:
                    st["evs"].append((kc, e))
                    return
                av_emit(st, kc, e)

            def av_emit(st, kc, e):
                vps = vp_pair[kc // 2][:, (kc % 2) * 128:(kc % 2 + 1) * 128]
                for h in range(2):
                    nc.tensor.matmul(
                        st["o"][h][:], lhsT=vps, rhs=e[:, h * 512:(h + 1) * 512],
                        start=(kc == 0), stop=(kc == NKC - 1),
                    )

            def pair_chunk(st, kc):
                s = st["sp"].tile([128, 1024], F32, tag="sp", name=f"s{st['p']}_{kc}")
                kslice = kpT_blk[kc // 4][:, (kc % 4) * 128:(kc % 4 + 1) * 128]
                for h in range(2):
                    nc.tensor.matmul(
                        s[:, h * 512:(h + 1) * 512], lhsT=kslice,
                        rhs=qpT_t[st["qs"][h]][:], start=True, stop=True,
                    )
                st["pend"].append((kc, s))
                if len(st["pend"]) > AV_STAGGER:
                    pair_drain(st)

            def pair_flush(st):
                while st["pend"]:
                    pair_drain(st)
                if st["defer"]:
                    # use the projection pool's banks (free after the kb
                    # loop) so the burst need not wait for pair 0's output
                    # copies to release the op slots
                    st["o"] = [pp.tile([128, 512], F32, tag="pp", name=f"o{q}")
                               for q in st["qs"]]
                    for kc, e in st["evs"]:
                        av_emit(st, kc, e)

            def pair_tail(st):
                for h, q in enumerate(st["qs"]):
                    ps_sum = st["sp"].tile([1, 512], F32, tag="sp", name=f"pssum{q}")
                    nc.tensor.matmul(
                        ps_sum[:], lhsT=ones_ap,
                        rhs=st["acc"][:, h * 512:(h + 1) * 512],
                        start=True, stop=True,
                    )
                    nc.scalar.activation(
                        sums_sb[:, q * 512:(q + 1) * 512], ps_sum[:], Copy
                    )
                    outsb = miscp.tile([128, 512], F32, tag="out", name=f"out{q}")
                    nc.vector.tensor_copy(outsb[:], st["o"][h][:])
                    nc.scalar.dma_start(outT[:, q * 512:(q + 1) * 512], outsb[:])

            # ---- kb loop: kp + vp projection, pair-0 attention interleaved ----
            st0 = pair_begin(0, sp)
            st1 = pair_begin(1, sp, defer_av=True)
            project_q(0)
            project_q(1)
            proj_k(0)
            for kb in range(NKB):
                if kb + 2 < NKB:
                    fetch_kx(kb + 2)
                if kb + 1 < NKB:
                    fetch_vx(kb + 1)
                    if kb > 0:
                        proj_k(kb + 1)

                vx = vxs.pop(kb)
                psv = pp.tile([128, 512], F32, tag="pp", name=f"psv{kb}")
                for c in range(NMC):
                    nc.tensor.matmul(
                        psv[:], lhsT=w_sb["wv"][:, c, :], rhs=vx[:, c, :],
                        start=(c == 0), stop=(c == NMC - 1),
                    )
                vpt = xs.tile([128, 512], BF16, tag="vpt", name=f"vpt{kb}")
                nc.vector.tensor_scalar_add(vpt[:], psv[:], bv_ap)
                for j in range(2):
                    tp = pp.tile([128, 256], BF16, tag="pp", name=f"tp{kb}_{j}")
                    for i in range(2):
                        nc.tensor.transpose(
                            tp[:, i * 128:(i + 1) * 128],
                            vpt[:, (2 * j + i) * 128:(2 * j + i + 1) * 128],
                            ident_sb[:],
                        )
                    nc.vector.tensor_copy(vp_pair[2 * kb + j][:], tp[:])

                if kb == 0:
                    pair_chunk(st0, 0)
                    pair_chunk(st0, 1)
                    proj_k(1)
                    pair_chunk(st0, 2)
                    pair_chunk(st0, 3)
                    project_q(2)
                    project_q(3)
                    for kc in range(4):
                        pair_chunk(st1, kc)
                else:
                    for kc in range(4 * kb, 4 * kb + 4):
                        pair_chunk(st0, kc)
                        pair_chunk(st1, kc)
            # early AV burst: everything already exp'd can run on the PE
            # while ScalarE finishes the exp-chain tail
            st1["o"] = [pp.tile([128, 512], F32, tag="pp", name=f"o{q}")
                        for q in st1["qs"]]
            for kc, e in st1["evs"]:
                av_emit(st1, kc, e)
            st1["evs"] = []
            pair_flush(st0)
            while st1["pend"]:
                pair_drain(st1)
            for kc, e in st1["evs"]:
                av_emit(st1, kc, e)
            pair_tail(st0)
            pair_tail(st1)
            nc.scalar.dma_start(aps["sums"][:], sums_sb[:])


_CACHE = {}


def _build():
    if "nc" in _CACHE:
        return _CACHE["nc"]
    nc = bacc.Bacc("TRN2", debug=False, num_devices=N_CORES)
    aps = {
        "qT": nc.dram_tensor("qT", [NQB, 128, NMC, 512], BF16,
                             kind="ExternalInput").ap(),
        "kT": nc.dram_tensor("kT", [NKB, 128, NMC, 512], BF16,
                             kind="ExternalInput").ap(),
        "vT": nc.dram_tensor("vT", [NKB, 128, NMC, 512], BF16,
                             kind="ExternalInput").ap(),
        "wq": nc.dram_tensor("wq", [128, NMC, DK], BF16, kind="ExternalInput").ap(),
        "wk": nc.dram_tensor("wk", [128, NMC, DK], BF16, kind="ExternalInput").ap(),
        "wv": nc.dram_tensor("wv", [128, NMC, DV], BF16, kind="ExternalInput").ap(),
        "bias_pack": nc.dram_tensor(
            "bias_pack", [128, 4], F32, kind="ExternalInput"
        ).ap(),
        "ident": nc.dram_tensor("ident", [128, 128], BF16, kind="ExternalInput").ap(),
        "outT": nc.dram_tensor("outT", [DV, SQ], F32, kind="ExternalOutput").ap(),
        "sums": nc.dram_tensor("sums", [1, SQ], F32, kind="ExternalOutput").ap(),
    }
    with tile.TileContext(nc) as tc:
        _emit(tc, aps)
    nc.compile()
    _CACHE["nc"] = nc
    return nc


def _pack_w(w):
    # [DM, d] -> [128, NMC, d]  (chunk-major weight layout)
    return np.ascontiguousarray(np.asarray(w).reshape(NMC, 128, -1).transpose(1, 0, 2))


def _pack_x(xT, nblk):
    # [DM, n] -> [nblk, 128, NMC, 512]  (contiguous per-stripe layout)
    return np.ascontiguousarray(
        xT.reshape(NMC, 128, nblk, 512).transpose(2, 1, 0, 3))


def make_in_maps(q, k, v, wq, bq, wk, bk, wv, bv):
    scale = 1.0 / math.sqrt(DK)
    wq_s = _pack_w((np.asarray(wq, np.float32) * scale).astype(NP_BF16))
    wk_b = _pack_w(np.asarray(wk, np.float32).astype(NP_BF16))
    wv_b = _pack_w(np.asarray(wv, np.float32).astype(NP_BF16))
    bias_pack = np.zeros((128, 4), np.float32)
    bias_pack[:, 0] = np.asarray(bq, np.float32) * scale
    bias_pack[:, 1] = np.asarray(bk, np.float32)
    bias_pack[:, 2] = np.asarray(bv, np.float32)
    bias_pack[:, 3] = 1.0
    ident = np.eye(128, dtype=NP_BF16)

    in_maps = []
    for core in range(N_CORES):
        b, h = core // 2, core % 2
        qTb = _pack_x(
            np.asarray(q[b], np.float32).T[:, h * SQ:(h + 1) * SQ].astype(NP_BF16),
            NQB)
        kTb = _pack_x(np.asarray(k[b], np.float32).T.astype(NP_BF16), NKB)
        vTb = _pack_x(np.asarray(v[b], np.float32).T.astype(NP_BF16), NKB)
        in_maps.append({
            "qT": qTb, "kT": kTb, "vT": vTb,
            "wq": wq_s, "wk": wk_b, "wv": wv_b,
            "bias_pack": bias_pack, "ident": ident,
        })
    return in_maps


def kernel(q, k, v, wq, bq, wk, bk, wv, bv, _trace=False, _tmpdir=None):
    nc = _build()
    in_maps = make_in_maps(q, k, v, wq, bq, wk, bk, wv, bv)
    res = run_bass_kernel_spmd(
        nc, in_maps, list(range(N_CORES)), trace=_trace, tmpdir=_tmpdir
    )
    out = np.empty((B, S, DV), np.float32)
    for core in range(N_CORES):
        b, h = core // 2, core % 2
        r = res.results[core]
        out[b, h * SQ:(h + 1) * SQ, :] = (r["outT"] / r["sums"]).T
    if _trace:
        kernel.last_results = res
    return out



# revision 4
# speedup vs baseline: 1.1128x; 1.1128x over previous
"""Bass/Trainium2 kernel for batched dot-product attention.

Problem: q,k,v [B=4, S=4096, D=1024]; projections to dk=dv=128; softmax
attention per batch element.  Sharded over 8 NeuronCores as (batch,
key-half): core c handles batch c//2 with keys (c%2)*2048 ... +2048 and
ALL 4096 queries, producing unnormalized partial AV outputs plus partial
exp-sums; the host merges the two key-halves (flash-attention style
without max subtraction -- scores are small, exp is safe in bf16).

Versus the query-split layout this deduplicates the kp/vp projections
(each key is projected once instead of twice); qp is duplicated instead,
which is half the size, and q/k projections run in fp8 (DoubleRow, 2x)
making the duplication cheap.

On-chip layouts keep the contraction dim on SBUF partitions:
  qT/kT/vT  [d_model, seq]   (host pre-transposed; q/k fp8e4, v bf16)
  qpT/kpT   [dk, seq]        (bf16)
  vp        [seq, dv]        (natural layout via PE transpose, bf16)
  S^T tiles [keys, q]        (scores transposed, PSUM f32)
  out^T     [dv, q]          (partial AV, bf16; host merges + undoes)

fp8 notes: wq/wk are boosted by a power of two before e4m3 quantization
(else they land in the subnormal range) and de-boosted in the fused
scale+bias step off PSUM.  Scale 1/sqrt(dk) is folded into wq/bq.

Softmax denominators: binary tree of bf16 adds over the 16 exp tiles of
each query block (DVE 2x mode), shipped as a [128, 4096] partial-sum
plane; host reduces the final 128 partitions.
"""

import math

import numpy as np
import ml_dtypes

import concourse.bass as bass
import concourse.tile as tile
from concourse import bacc, mybir
from concourse.bass_utils import run_bass_kernel_spmd

B, S, DM, DK, DV = 4, 4096, 1024, 128, 128
N_CORES = 8
KH = S // 2          # keys per core (2048)
NKC = KH // 128      # key chunks of 128 per core (16)
NKB = KH // 512      # key blocks of 512 per core (4)
NQB = S // 1024      # query blocks of 1024 (4)
NQPB = S // 512      # qp projection blocks of 512 (8)
NMC = DM // 128      # d_model chunks (8)

FP8_QK = True        # q/k projections in fp8e4 DoubleRow (2x PE, half DMA)
WQ_BOOST = 128.0     # power-of-2 pre-quantization boost for wq*scale
WK_BOOST = 32.0      # same for wk

BF16 = mybir.dt.bfloat16
F32 = mybir.dt.float32
E4 = mybir.dt.float8e4
NP_BF16 = ml_dtypes.bfloat16
NP_E4 = ml_dtypes.float8_e4m3fn

QK_DT = E4 if FP8_QK else BF16
NP_QK = NP_E4 if FP8_QK else NP_BF16

Exp = mybir.ActivationFunctionType.Exp
DoubleRow = mybir.MatmulPerfMode.DoubleRow


def _emit(tc: tile.TileContext, aps: dict):
    nc = tc.nc
    qT, kT, vT = aps["qT"], aps["kT"], aps["vT"]
    outT, accT = aps["outT"], aps["accT"]

    with tc.tile_pool(name="persist", bufs=1) as persist:
        # --- constants (scalar queue, so they never block the input stream)
        w_sb = {}
        for name, dt in (("wq", QK_DT), ("wk", QK_DT), ("wv", BF16)):
            t = persist.tile([128, NMC, 128], dt, tag=f"w_{name}", name=f"w_{name}")
            nc.scalar.dma_start(t[:], aps[name][:])
            w_sb[name] = t
        bias_sb = persist.tile([128, 4], F32, tag="bias")
        nc.scalar.dma_start(bias_sb[:], aps["bias_pack"][:])
        bq_ap, bk_ap, bv_ap = bias_sb[:, 0:1], bias_sb[:, 1:2], bias_sb[:, 2:3]
        ident_sb = persist.tile([128, 128], BF16, tag="ident")
        nc.scalar.dma_start(ident_sb[:], aps["ident"][:])

        # --- persistent activations ---
        kpT = persist.tile([128, NKB, 512], BF16, tag="kpT", name="kpT")
        qpT = persist.tile([128, NQPB, 512], BF16, tag="qpT", name="qpT")
        vp = persist.tile([128, NKC, 128], BF16, tag="vp", name="vp")

        with (
            tc.tile_pool(name="pp", bufs=2, space="PSUM") as pp,
            tc.tile_pool(name="sp", bufs=2, space="PSUM") as sp,
            tc.tile_pool(name="ovp", bufs=1, space="PSUM") as ovp,
            tc.tile_pool(name="xs", bufs=2) as xs,
            tc.tile_pool(name="ep", bufs=6) as ep,
            tc.tile_pool(name="trp", bufs=2) as trp,
            tc.tile_pool(name="outp", bufs=2) as outp,
        ):
            # ---- input fetch (gpsimd queue: near-free issue) ----
            kxs, vxs, qxs = {}, {}, {}

            def fetch(store, src, blk, dt, tag, bufs, parts):
                t = xs.tile([128, NMC, 512], dt, tag=tag, name=f"{tag}{blk}",
                            bufs=bufs)
                step = NMC // parts
                for i in range(0, NMC, step):
                    nc.gpsimd.dma_start(t[:, i:i + step, :],
                                        src[blk][:, i:i + step, :])
                store[blk] = t

            def fetch_kx(blk, parts=2):
                fetch(kxs, kT, blk, QK_DT, "kx", 2, parts)

            def fetch_qx(blk, parts=2):
                fetch(qxs, qT, blk, QK_DT, "qx", 3, parts)

            def fetch_vx(blk, parts=2):
                fetch(vxs, vT, blk, BF16, "vx", 2, parts)

            # ---- projections ----
            def proj_fp8(w, x, ps, c0, c1):
                for c in range(c0, c1, 2):
                    nc.tensor.matmul(
                        ps[:], lhsT=w[:, c:c + 2, :], rhs=x[:, c:c + 2, :],
                        start=(c == 0), stop=(c == NMC - 2),
                        perf_mode=DoubleRow,
                    )

            def proj_bf16(w, x, ps, c0, c1):
                for c in range(c0, c1):
                    nc.tensor.matmul(
                        ps[:], lhsT=w[:, c, :], rhs=x[:, c, :],
                        start=(c == 0), stop=(c == NMC - 1),
                    )

            proj_qk = proj_fp8 if FP8_QK else proj_bf16

            def proj_qp(qb, c0=0, c1=NMC, _ps={}):
                if c0 == 0:
                    _ps[qb] = pp.tile([128, 512], F32, tag="pp", name=f"psq{qb}")
                proj_qk(w_sb["wq"], qxs[qb][:], _ps[qb][:], c0, c1)
                if c1 == NMC:
                    qxs.pop(qb)
                    if FP8_QK:
                        nc.vector.tensor_scalar(
                            qpT[:, qb, :], _ps.pop(qb)[:], 1.0 / WQ_BOOST,
                            bq_ap, mybir.AluOpType.mult, mybir.AluOpType.add)
                    else:
                        nc.vector.tensor_scalar_add(
                            qpT[:, qb, :], _ps.pop(qb)[:], bq_ap)

            def proj_kp(kb, c0=0, c1=NMC, _ps={}):
                if c0 == 0:
                    _ps[kb] = pp.tile([128, 512], F32, tag="pp", name=f"psk{kb}")
                proj_qk(w_sb["wk"], kxs[kb][:], _ps[kb][:], c0, c1)
                if c1 == NMC:
                    kxs.pop(kb)
                    if FP8_QK:
                        nc.vector.tensor_scalar(
                            kpT[:, kb, :], _ps.pop(kb)[:], 1.0 / WK_BOOST,
                            bk_ap, mybir.AluOpType.mult, mybir.AluOpType.add)
                    else:
                        nc.vector.tensor_scalar_add(
                            kpT[:, kb, :], _ps.pop(kb)[:], bk_ap)

            def proj_vp(kb):
                vx = vxs.pop(kb)
                psv = pp.tile([128, 512], F32, tag="pp", name=f"psv{kb}")
                proj_bf16(w_sb["wv"], vx[:], psv[:], 0, NMC)
                vpt = xs.tile([128, 512], BF16, tag="vpt", name=f"vpt{kb}")
                nc.vector.tensor_scalar_add(vpt[:], psv[:], bv_ap)
                for j in range(2):
                    tp = pp.tile([128, 256], BF16, tag="pp", name=f"tp{kb}_{j}")
                    for i in range(2):
                        nc.tensor.transpose(
                            tp[:, i * 128:(i + 1) * 128],
                            vpt[:, (2 * j + i) * 128:(2 * j + i + 1) * 128],
                            ident_sb[:],
                        )
                    nc.vector.tensor_copy(vp[:, 4 * kb + 2 * j:4 * kb + 2 * j + 2, :],
                                          tp[:])

            # ---- attention machinery (per query block of 1024) ----
            class QB:
                def __init__(self, qb):
                    self.qb = qb
                    self.ov = ovp.tile([128, 1024], F32, tag="ov", name=f"ov{qb}")
                    self.pend = []       # (kc, s_psum) awaiting exp
                    self.av_pend = []    # (kc, e) awaiting AV matmul
                    self.levels = [None] * 5

                def scores(self, kc):
                    s = sp.tile([128, 1024], F32, tag="sp",
                                name=f"s{self.qb}_{kc}")
                    ks = kpT[:, kc // 4, (kc % 4) * 128:(kc % 4 + 1) * 128]
                    for h in range(2):
                        nc.tensor.matmul(
                            s[:, h * 512:(h + 1) * 512], lhsT=ks,
                            rhs=qpT[:, 2 * self.qb + h, :],
                            start=True, stop=True,
                        )
                    self.pend.append((kc, s))

                def exp(self):
                    kc, s = self.pend.pop(0)
                    e = ep.tile([128, 1024], BF16, tag="e",
                                name=f"e{self.qb}_{kc}")
                    nc.scalar.activation(e[:], s[:], Exp)
                    self.av_pend.append((kc, e))
                    self._tree_feed(kc, e)

                def av(self):
                    kc, e = self.av_pend.pop(0)
                    for h in range(2):
                        nc.tensor.matmul(
                            self.ov[:, h * 512:(h + 1) * 512],
                            lhsT=vp[:, kc, :],
                            rhs=e[:, h * 512:(h + 1) * 512],
                            start=(kc == 0), stop=(kc == NKC - 1),
                        )

                def _tree_feed(self, kc, e):
                    cur, lvl = e, 0
                    while self.levels[lvl] is not None:
                        prev = self.levels[lvl]
                        self.levels[lvl] = None
                        nt = trp.tile([128, 1024], BF16, tag=f"t{lvl}",
                                      name=f"t{self.qb}_{kc}_{lvl}", bufs=2)
                        nc.vector.tensor_add(nt[:], prev[:], cur[:])
                        cur, lvl = nt, lvl + 1
                    self.levels[lvl] = cur

                def drain(self):
                    acc = self.levels[4]
                    assert acc is not None
                    nc.sync.dma_start(
                        accT[:, self.qb * 1024:(self.qb + 1) * 1024], acc[:])
                    outsb = outp.tile([128, 1024], BF16, tag="out",
                                      name=f"out{self.qb}")
                    nc.vector.tensor_copy(outsb[:], self.ov[:])
                    nc.sync.dma_start(
                        outT[:, self.qb * 1024:(self.qb + 1) * 1024], outsb[:])

            # ---- schedule ----
            # Input stream: kx0/qx0 lead finely chunked so the first
            # projection matmuls start as soon as data lands.
            fetch_kx(0, parts=4)
            fetch_qx(0, parts=4)
            fetch_vx(0)
            fetch_kx(1)
            fetch_qx(1)
            fetch_vx(1)
            fetch_kx(2)
            fetch_qx(2)

            proj_kp(0)
            proj_qp(0)
            proj_vp(0)
            proj_qp(1)

            # Detours: PE work emitted between attention chunks of qb0/qb1.
            # Each entry runs after that kc's scores+exp are emitted.
            half = NMC // 2
            detours = {
                (0, 1): [lambda: fetch_vx(2), lambda: fetch_kx(3)],
                (0, 2): [lambda: proj_kp(1, 0, half)],
                (0, 3): [lambda: proj_kp(1, half, NMC), lambda: fetch_qx(3)],
                (0, 4): [lambda: proj_vp(1)],
                (0, 6): [lambda: proj_kp(2, 0, half), lambda: fetch_vx(3)],
                (0, 7): [lambda: proj_kp(2, half, NMC), lambda: fetch_qx(4)],
                (0, 8): [lambda: proj_vp(2)],
                (0, 10): [lambda: proj_kp(3, 0, half), lambda: fetch_qx(5)],
                (0, 11): [lambda: proj_kp(3, half, NMC)],
                (0, 12): [lambda: proj_vp(3), lambda: fetch_qx(6)],
                (0, 13): [lambda: proj_qp(2)],
                (0, 14): [lambda: proj_qp(3), lambda: fetch_qx(7)],
                (1, 0): [lambda: proj_qp(4)],
                (1, 2): [lambda: proj_qp(5)],
                (1, 4): [lambda: proj_qp(6)],
                (1, 6): [lambda: proj_qp(7)],
            }

            prev = None
            for qb in range(NQB):
                st = QB(qb)
                for kc in range(NKC):
                    st.scores(kc)
                    st.exp()
                    for d in detours.get((qb, kc), ()):
                        d()
                    if kc >= 1:
                        st.av()
                    if kc == 0 and prev is not None:
                        prev.drain()
                        prev = None
                st.av()
                prev = st
            prev.drain()


_CACHE = {}


def _build():
    if "nc" in _CACHE:
        return _CACHE["nc"]
    nc = bacc.Bacc("TRN2", debug=False, num_devices=N_CORES)
    aps = {
        "qT": nc.dram_tensor("qT", [NQPB, 128, NMC, 512], QK_DT,
                             kind="ExternalInput").ap(),
        "kT": nc.dram_tensor("kT", [NKB, 128, NMC, 512], QK_DT,
                             kind="ExternalInput").ap(),
        "vT": nc.dram_tensor("vT", [NKB, 128, NMC, 512], BF16,
                             kind="ExternalInput").ap(),
        "wq": nc.dram_tensor("wq", [128, NMC, DK], QK_DT,
                             kind="ExternalInput").ap(),
        "wk": nc.dram_tensor("wk", [128, NMC, DK], QK_DT,
                             kind="ExternalInput").ap(),
        "wv": nc.dram_tensor("wv", [128, NMC, DV], BF16,
                             kind="ExternalInput").ap(),
        "bias_pack": nc.dram_tensor(
            "bias_pack", [128, 4], F32, kind="ExternalInput").ap(),
        "ident": nc.dram_tensor("ident", [128, 128], BF16,
                                kind="ExternalInput").ap(),
        "outT": nc.dram_tensor("outT", [DV, S], BF16,
                               kind="ExternalOutput").ap(),
        "accT": nc.dram_tensor("accT", [128, S], BF16,
                               kind="ExternalOutput").ap(),
    }
    with tile.TileContext(nc) as tc:
        _emit(tc, aps)
    nc.compile()
    _CACHE["nc"] = nc
    return nc


def _pack_w(w, np_dt):
    # [DM, d] -> [128, NMC, d]  (chunk-major weight layout)
    return np.ascontiguousarray(
        np.asarray(w).reshape(NMC, 128, -1).transpose(1, 0, 2)).astype(np_dt)


def _pack_x(xT, nblk, np_dt):
    # [DM, n] -> [nblk, 128, NMC, 512]  (contiguous per-stripe layout)
    return np.ascontiguousarray(
        xT.reshape(NMC, 128, nblk, 512).transpose(2, 1, 0, 3)).astype(np_dt)


def make_in_maps(q, k, v, wq, bq, wk, bk, wv, bv):
    scale = 1.0 / math.sqrt(DK)
    if FP8_QK:
        wq_p = _pack_w(np.asarray(wq, np.float32) * (scale * WQ_BOOST), NP_QK)
        wk_p = _pack_w(np.asarray(wk, np.float32) * WK_BOOST, NP_QK)
    else:
        wq_p = _pack_w(np.asarray(wq, np.float32) * scale, NP_QK)
        wk_p = _pack_w(np.asarray(wk, np.float32), NP_QK)
    wv_p = _pack_w(np.asarray(wv, np.float32), NP_BF16)
    bias_pack = np.zeros((128, 4), np.float32)
    bias_pack[:, 0] = np.asarray(bq, np.float32) * scale
    bias_pack[:, 1] = np.asarray(bk, np.float32)
    bias_pack[:, 2] = np.asarray(bv, np.float32)
    ident = np.eye(128, dtype=NP_BF16)

    in_maps = []
    qT_cache = {}
    for core in range(N_CORES):
        b, h = core // 2, core % 2
        if b not in qT_cache:
            qT_cache[b] = _pack_x(np.asarray(q[b], np.float32).T, NQPB, NP_QK)
        kTb = _pack_x(
            np.asarray(k[b], np.float32).T[:, h * KH:(h + 1) * KH], NKB, NP_QK)
        vTb = _pack_x(
            np.asarray(v[b], np.float32).T[:, h * KH:(h + 1) * KH], NKB, NP_BF16)
        in_maps.append({
            "qT": qT_cache[b], "kT": kTb, "vT": vTb,
            "wq": wq_p, "wk": wk_p, "wv": wv_p,
            "bias_pack": bias_pack, "ident": ident,
        })
    return in_maps


def kernel(q, k, v, wq, bq, wk, bk, wv, bv, _trace=False, _tmpdir=None):
    nc = _build()
    in_maps = make_in_maps(q, k, v, wq, bq, wk, bk, wv, bv)
    res = run_bass_kernel_spmd(
        nc, in_maps, list(range(N_CORES)), trace=_trace, tmpdir=_tmpdir
    )
    out = np.empty((B, S, DV), np.float32)
    for b in range(B):
        r0, r1 = res.results[2 * b], res.results[2 * b + 1]
        o = r0["outT"].astype(np.float32) + r1["outT"].astype(np.float32)
        sums = (r0["accT"].astype(np.float32).sum(axis=0)
                + r1["accT"].astype(np.float32).sum(axis=0))
        out[b] = (o / sums[None, :]).T
    if _trace:
        kernel.last_results = res
    return out


# revision 8
# speedup vs baseline: 1.1240x; 1.0101x over previous
"""Bass/Trainium2 kernel for batched dot-product attention.

Problem: q,k,v [B=4, S=4096, D=1024]; projections to dk=dv=128; softmax
attention per batch element.  Sharded over 8 NeuronCores as (batch,
key-half): core c handles batch c//2 with keys (c%2)*2048 ... +2048 and
ALL 4096 queries, producing unnormalized partial AV outputs plus partial
exp-sums; the host merges the two key-halves (flash-attention style
without max subtraction -- scores are small, exp is safe in bf16).

Versus the query-split layout this deduplicates the kp/vp projections
(each key is projected once instead of twice); qp is duplicated instead,
which is half the size, and q/k projections run in fp8 (DoubleRow, 2x)
making the duplication cheap.

On-chip layouts keep the contraction dim on SBUF partitions:
  qT/kT/vT  [d_model, seq]   (host pre-transposed; q/k fp8e4, v bf16)
  qpT/kpT   [dk, seq]        (bf16)
  vp        [seq, dv]        (natural layout via PE transpose, bf16)
  S^T tiles [keys, q]        (scores transposed, PSUM f32)
  out^T     [dv, q]          (partial AV, bf16; host merges + undoes)

fp8 notes: wq/wk are boosted by a power of two before e4m3 quantization
(else they land in the subnormal range) and de-boosted in the fused
scale+bias step off PSUM.  Scale 1/sqrt(dk) is folded into wq/bq.

Softmax denominators: binary tree of bf16 adds over the 16 exp tiles of
each query block (DVE 2x mode), shipped as a [128, 4096] partial-sum
plane; host reduces the final 128 partitions.
"""

import math

import numpy as np
import ml_dtypes

import concourse.bass as bass
import concourse.tile as tile
from concourse import bacc, mybir
from concourse.bass_utils import run_bass_kernel_spmd

B, S, DM, DK, DV = 4, 4096, 1024, 128, 128
N_CORES = 8
KH = S // 2          # keys per core (2048)
NKC = KH // 128      # key chunks of 128 per core (16)
NKB = KH // 512      # key blocks of 512 per core (4)
NQB = S // 1024      # query blocks of 1024 (4)
NQPB = S // 512      # qp projection blocks of 512 (8)
NMC = DM // 128      # d_model chunks (8)

FP8_QK = True        # q/k projections in fp8e4 DoubleRow (2x PE, half DMA)
WQ_BOOST = 128.0     # power-of-2 pre-quantization boost for wq*scale
WK_BOOST = 32.0      # same for wk

BF16 = mybir.dt.bfloat16
F32 = mybir.dt.float32
E4 = mybir.dt.float8e4
NP_BF16 = ml_dtypes.bfloat16
NP_E4 = ml_dtypes.float8_e4m3fn

QK_DT = E4 if FP8_QK else BF16
NP_QK = NP_E4 if FP8_QK else NP_BF16

Exp = mybir.ActivationFunctionType.Exp
DoubleRow = mybir.MatmulPerfMode.DoubleRow


def _emit(tc: tile.TileContext, aps: dict):
    nc = tc.nc
    qT, kT, vT = aps["qT"], aps["kT"], aps["vT"]
    outT, accT = aps["outT"], aps["accT"]

    with tc.tile_pool(name="persist", bufs=1) as persist:
        # --- constants (scalar queue, so they never block the input stream)
        w_sb = {}
        for name, dt in (("wq", QK_DT), ("wk", QK_DT), ("wv", BF16)):
            t = persist.tile([128, NMC, 128], dt, tag=f"w_{name}", name=f"w_{name}")
            nc.scalar.dma_start(t[:], aps[name][:])
            w_sb[name] = t
        bias_sb = persist.tile([128, 4], F32, tag="bias")
        nc.scalar.dma_start(bias_sb[:], aps["bias_pack"][:])
        bq_ap, bk_ap, bv_ap = bias_sb[:, 0:1], bias_sb[:, 1:2], bias_sb[:, 2:3]
        ident_sb = persist.tile([128, 128], BF16, tag="ident")
        nc.scalar.dma_start(ident_sb[:], aps["ident"][:])

        # --- persistent activations ---
        kpT = persist.tile([128, NKB, 512], BF16, tag="kpT", name="kpT")
        qpT = persist.tile([128, NQPB, 512], BF16, tag="qpT", name="qpT")
        vp = persist.tile([128, NKC, 128], BF16, tag="vp", name="vp")

        with (
            tc.tile_pool(name="pp", bufs=2, space="PSUM") as pp,
            tc.tile_pool(name="sp", bufs=2, space="PSUM") as sp,
            tc.tile_pool(name="ovp", bufs=1, space="PSUM") as ovp,
            tc.tile_pool(name="xs", bufs=2) as xs,
            tc.tile_pool(name="ep", bufs=6) as ep,
            tc.tile_pool(name="trp", bufs=2) as trp,
            tc.tile_pool(name="outp", bufs=2) as outp,
        ):
            # ---- input fetch (gpsimd queue: near-free issue) ----
            kxs, vxs, qxs = {}, {}, {}

            def fetch(store, src, blk, dt, tag, bufs, parts):
                t = xs.tile([128, NMC, 512], dt, tag=tag, name=f"{tag}{blk}",
                            bufs=bufs)
                step = NMC // parts
                for i in range(0, NMC, step):
                    nc.gpsimd.dma_start(t[:, i:i + step, :],
                                        src[blk][:, i:i + step, :])
                store[blk] = t

            def fetch_kx(blk, parts=2):
                fetch(kxs, kT, blk, QK_DT, "kx", 2, parts)

            def fetch_qx(blk, parts=2):
                fetch(qxs, qT, blk, QK_DT, "qx", 3, parts)

            def fetch_vx(blk, parts=2):
                fetch(vxs, vT, blk, BF16, "vx", 2, parts)

            # ---- projections ----
            def proj_fp8(w, x, ps, c0, c1):
                for c in range(c0, c1, 2):
                    nc.tensor.matmul(
                        ps[:], lhsT=w[:, c:c + 2, :], rhs=x[:, c:c + 2, :],
                        start=(c == 0), stop=(c == NMC - 2),
                        perf_mode=DoubleRow,
                    )

            def proj_bf16(w, x, ps, c0, c1):
                for c in range(c0, c1):
                    nc.tensor.matmul(
                        ps[:], lhsT=w[:, c, :], rhs=x[:, c, :],
                        start=(c == 0), stop=(c == NMC - 1),
                    )

            proj_qk = proj_fp8 if FP8_QK else proj_bf16

            def proj_qp(qb, c0=0, c1=NMC, _ps={}):
                if c0 == 0:
                    _ps[qb] = pp.tile([128, 512], F32, tag="pp", name=f"psq{qb}")
                proj_qk(w_sb["wq"], qxs[qb][:], _ps[qb][:], c0, c1)
                if c1 == NMC:
                    qxs.pop(qb)
                    if FP8_QK:
                        nc.vector.tensor_scalar(
                            qpT[:, qb, :], _ps.pop(qb)[:], 1.0 / WQ_BOOST,
                            bq_ap, mybir.AluOpType.mult, mybir.AluOpType.add)
                    else:
                        nc.vector.tensor_scalar_add(
                            qpT[:, qb, :], _ps.pop(qb)[:], bq_ap)

            def proj_kp(kb, c0=0, c1=NMC, _ps={}):
                if c0 == 0:
                    _ps[kb] = pp.tile([128, 512], F32, tag="pp", name=f"psk{kb}")
                proj_qk(w_sb["wk"], kxs[kb][:], _ps[kb][:], c0, c1)
                if c1 == NMC:
                    kxs.pop(kb)
                    if FP8_QK:
                        nc.vector.tensor_scalar(
                            kpT[:, kb, :], _ps.pop(kb)[:], 1.0 / WK_BOOST,
                            bk_ap, mybir.AluOpType.mult, mybir.AluOpType.add)
                    else:
                        nc.vector.tensor_scalar_add(
                            kpT[:, kb, :], _ps.pop(kb)[:], bk_ap)

            def proj_vp(kb, c0=0, c1=NMC, _ps={}):
                if c0 == 0:
                    _ps[kb] = pp.tile([128, 512], F32, tag="pp", name=f"psv{kb}")
                proj_bf16(w_sb["wv"], vxs[kb][:], _ps[kb][:], c0, c1)
                if c1 < NMC:
                    return
                vxs.pop(kb)
                vpt = xs.tile([128, 512], BF16, tag="vpt", name=f"vpt{kb}")
                nc.vector.tensor_scalar_add(vpt[:], _ps.pop(kb)[:], bv_ap)
                for j in range(2):
                    tp = pp.tile([128, 256], BF16, tag="pp", name=f"tp{kb}_{j}")
                    for i in range(2):
                        nc.tensor.transpose(
                            tp[:, i * 128:(i + 1) * 128],
                            vpt[:, (2 * j + i) * 128:(2 * j + i + 1) * 128],
                            ident_sb[:],
                        )
                    nc.vector.tensor_copy(vp[:, 4 * kb + 2 * j:4 * kb + 2 * j + 2, :],
                                          tp[:])

            # ---- attention machinery (per query block of 1024) ----
            class QB:
                def __init__(self, qb):
                    self.qb = qb
                    self.ov = ovp.tile([128, 1024], F32, tag="ov", name=f"ov{qb}")
                    self.pend = []       # (kc, s_psum) awaiting exp
                    self.av_pend = []    # (kc, e) awaiting AV matmul
                    self.acc = None      # running bf16 sum of exp tiles

                def scores(self, kc, halves=(0, 1)):
                    if not self.pend or self.pend[-1][0] != kc:
                        s = sp.tile([128, 1024], F32, tag="sp",
                                    name=f"s{self.qb}_{kc}")
                        self.pend.append((kc, s))
                    s = self.pend[-1][1]
                    ks = kpT[:, kc // 4, (kc % 4) * 128:(kc % 4 + 1) * 128]
                    for h in halves:
                        nc.tensor.matmul(
                            s[:, h * 512:(h + 1) * 512], lhsT=ks,
                            rhs=qpT[:, 2 * self.qb + h, :],
                            start=True, stop=True,
                        )

                def exp(self, halves=None):
                    kc, s = self.pend.pop(0)
                    e = ep.tile([128, 1024], BF16, tag="e",
                                name=f"e{self.qb}_{kc}")
                    if halves is None:
                        nc.scalar.activation(e[:], s[:], Exp)
                    else:
                        for h in halves:
                            nc.scalar.activation(
                                e[:, h * 512:(h + 1) * 512],
                                s[:, h * 512:(h + 1) * 512], Exp)
                    self.av_pend.append((kc, e))
                    if self.acc is None:
                        self.acc = e
                    else:
                        nt = trp.tile([128, 1024], BF16, tag="chain",
                                      name=f"c{self.qb}_{kc}", bufs=2)
                        nc.vector.tensor_add(nt[:], self.acc[:], e[:])
                        self.acc = nt

                def av(self):
                    kc, e = self.av_pend.pop(0)
                    for h in range(2):
                        nc.tensor.matmul(
                            self.ov[:, h * 512:(h + 1) * 512],
                            lhsT=vp[:, kc, :],
                            rhs=e[:, h * 512:(h + 1) * 512],
                            start=(kc == 0), stop=(kc == NKC - 1),
                        )

                def drain(self, split=False):
                    nc.sync.dma_start(
                        accT[:, self.qb * 1024:(self.qb + 1) * 1024],
                        self.acc[:])
                    outsb = outp.tile([128, 1024], BF16, tag="out",
                                      name=f"out{self.qb}")
                    qlo = self.qb * 1024
                    if split:
                        for h in range(2):
                            nc.vector.tensor_copy(
                                outsb[:, h * 512:(h + 1) * 512],
                                self.ov[:, h * 512:(h + 1) * 512])
                            nc.sync.dma_start(
                                outT[:, qlo + h * 512:qlo + (h + 1) * 512],
                                outsb[:, h * 512:(h + 1) * 512])
                    else:
                        nc.vector.tensor_copy(outsb[:], self.ov[:])
                        nc.sync.dma_start(
                            outT[:, qlo:qlo + 1024], outsb[:])

            # ---- schedule ----
            # Input stream: kx0/qx0/qx1 lead finely chunked so the first
            # scores+exp start as soon as data lands; vx0 follows.
            fetch_kx(0, parts=4)
            fetch_qx(0, parts=4)
            fetch_qx(1)
            fetch_vx(0)
            fetch_kx(1)
            fetch_vx(1)
            fetch_kx(2)
            fetch_vx(2)
            fetch_kx(3)
            fetch_vx(3)
            for qb in range(2, NQPB):
                fetch_qx(qb)

            # Critical prefix: first exp fires after kp0 + qp0 (h0 half)
            # then qp1 completes the tile.
            st0 = QB(0)
            proj_kp(0)
            proj_qp(0)
            st0.scores(0, halves=(0,))
            proj_qp(1)
            st0.scores(0, halves=(1,))
            st0.exp(halves=(0, 1))
            st0.scores(1)
            st0.exp()
            proj_vp(0)

            # Detours: PE work emitted between attention chunks of qb0/qb1,
            # each at most ~1.8us so the 2-deep scores lookahead keeps the
            # exp chain fed.
            half = NMC // 2
            detours = {
                (0, 2): [lambda: proj_kp(1, 0, half)],
                (0, 3): [lambda: proj_kp(1, half, NMC)],
                (0, 4): [lambda: proj_vp(1, 0, half)],
                (0, 5): [lambda: proj_vp(1, half, NMC)],
                (0, 6): [lambda: proj_kp(2, 0, half)],
                (0, 7): [lambda: proj_kp(2, half, NMC)],
                (0, 8): [lambda: proj_vp(2, 0, half)],
                (0, 9): [lambda: proj_vp(2, half, NMC)],
                (0, 10): [lambda: proj_kp(3, 0, half)],
                (0, 11): [lambda: proj_kp(3, half, NMC)],
                (0, 12): [lambda: proj_vp(3, 0, half)],
                (0, 13): [lambda: proj_vp(3, half, NMC)],
                (0, 14): [lambda: proj_qp(2)],
                (0, 15): [lambda: proj_qp(3)],
                (1, 0): [lambda: proj_qp(4)],
                (1, 2): [lambda: proj_qp(5)],
                (1, 4): [lambda: proj_qp(6)],
                (1, 6): [lambda: proj_qp(7)],
            }

            prev = None
            for qb in range(NQB):
                st = st0 if qb == 0 else QB(qb)
                for kc in range(2 if qb == 0 else 0, NKC):
                    st.scores(kc)
                    st.exp()
                    for d in detours.get((qb, kc), ()):
                        d()
                    if kc >= 1:
                        st.av()
                    if kc == 0 and prev is not None:
                        prev.drain()
                        prev = None
                while st.av_pend:
                    st.av()
                prev = st
            prev.drain(split=True)


_CACHE = {}


def _build():
    if "nc" in _CACHE:
        return _CACHE["nc"]
    nc = bacc.Bacc("TRN2", debug=False, num_devices=N_CORES)
    aps = {
        "qT": nc.dram_tensor("qT", [NQPB, 128, NMC, 512], QK_DT,
                             kind="ExternalInput").ap(),
        "kT": nc.dram_tensor("kT", [NKB, 128, NMC, 512], QK_DT,
                             kind="ExternalInput").ap(),
        "vT": nc.dram_tensor("vT", [NKB, 128, NMC, 512], BF16,
                             kind="ExternalInput").ap(),
        "wq": nc.dram_tensor("wq", [128, NMC, DK], QK_DT,
                             kind="ExternalInput").ap(),
        "wk": nc.dram_tensor("wk", [128, NMC, DK], QK_DT,
                             kind="ExternalInput").ap(),
        "wv": nc.dram_tensor("wv", [128, NMC, DV], BF16,
                             kind="ExternalInput").ap(),
        "bias_pack": nc.dram_tensor(
            "bias_pack", [128, 4], F32, kind="ExternalInput").ap(),
        "ident": nc.dram_tensor("ident", [128, 128], BF16,
                                kind="ExternalInput").ap(),
        "outT": nc.dram_tensor("outT", [DV, S], BF16,
                               kind="ExternalOutput").ap(),
        "accT": nc.dram_tensor("accT", [128, S], BF16,
                               kind="ExternalOutput").ap(),
    }
    with tile.TileContext(nc) as tc:
        _emit(tc, aps)
    nc.compile()
    _CACHE["nc"] = nc
    return nc


def _pack_w(w, np_dt):
    # [DM, d] -> [128, NMC, d]  (chunk-major weight layout)
    return np.ascontiguousarray(
        np.asarray(w).reshape(NMC, 128, -1).transpose(1, 0, 2)).astype(np_dt)


def _pack_x(xT, nblk, np_dt):
    # [DM, n] -> [nblk, 128, NMC, 512]  (contiguous per-stripe layout)
    return np.ascontiguousarray(
        xT.reshape(NMC, 128, nblk, 512).transpose(2, 1, 0, 3)).astype(np_dt)


def make_in_maps(q, k, v, wq, bq, wk, bk, wv, bv):
    scale = 1.0 / math.sqrt(DK)
    if FP8_QK:
        wq_p = _pack_w(np.asarray(wq, np.float32) * (scale * WQ_BOOST), NP_QK)
        wk_p = _pack_w(np.asarray(wk, np.float32) * WK_BOOST, NP_QK)
    else:
        wq_p = _pack_w(np.asarray(wq, np.float32) * scale, NP_QK)
        wk_p = _pack_w(np.asarray(wk, np.float32), NP_QK)
    wv_p = _pack_w(np.asarray(wv, np.float32), NP_BF16)
    bias_pack = np.zeros((128, 4), np.float32)
    bias_pack[:, 0] = np.asarray(bq, np.float32) * scale
    bias_pack[:, 1] = np.asarray(bk, np.float32)
    bias_pack[:, 2] = np.asarray(bv, np.float32)
    ident = np.eye(128, dtype=NP_BF16)

    in_maps = []
    qT_cache = {}
    for core in range(N_CORES):
        b, h = core // 2, core % 2
        if b not in qT_cache:
            qT_cache[b] = _pack_x(np.asarray(q[b], np.float32).T, NQPB, NP_QK)
        kTb = _pack_x(
            np.asarray(k[b], np.float32).T[:, h * KH:(h + 1) * KH], NKB, NP_QK)
        vTb = _pack_x(
            np.asarray(v[b], np.float32).T[:, h * KH:(h + 1) * KH], NKB, NP_BF16)
        in_maps.append({
            "qT": qT_cache[b], "kT": kTb, "vT": vTb,
            "wq": wq_p, "wk": wk_p, "wv": wv_p,
            "bias_pack": bias_pack, "ident": ident,
        })
    return in_maps


def kernel(q, k, v, wq, bq, wk, bk, wv, bv, _trace=False, _tmpdir=None):
    nc = _build()
    in_maps = make_in_maps(q, k, v, wq, bq, wk, bk, wv, bv)
    res = run_bass_kernel_spmd(
        nc, in_maps, list(range(N_CORES)), trace=_trace, tmpdir=_tmpdir
    )
    out = np.empty((B, S, DV), np.float32)
    for b in range(B):
        r0, r1 = res.results[2 * b], res.results[2 * b + 1]
        o = r0["outT"].astype(np.float32) + r1["outT"].astype(np.float32)
        sums = (r0["accT"].astype(np.float32).sum(axis=0)
                + r1["accT"].astype(np.float32).sum(axis=0))
        out[b] = (o / sums[None, :]).T
    if _trace:
        kernel.last_results = res
    return out
